# revision 8
# baseline (speedup 1.0000x reference)
"""Tensor-parallel GQA attention (Llama-3-8B shape, prefill, start_pos=0) on 8
Trainium2 NeuronCores.

Sharding: core i owns kv-head i and q-heads 4i..4i+3 — wq/wk/wv column-shards,
wo row-shard, x replicated.  Each core computes a partial [2048, 4096] output
(its heads pushed through its wo rows); the host sums the 8 partials
(all-reduce equivalent).

Per-core kernel layout (matmuls N=512, bf16 operands, fp32 PSUM accumulate):
  - xT [D, S] on device; projections computed with weights as the stationary
    operand, giving qT/kT/vT in [head_dim, seq] layout directly.
  - RoPE in [head_dim, seq] layout; rotate-half is a PE matmul against a
    signed permutation matrix (keeps the DVE FIFO free of DMA waits), the
    1/sqrt(hd) score scale is folded into q's sin/cos tables.
  - Scores computed transposed, ST[j, i] = kT.T @ qT, so exp needs no
    transposes and PV consumes Pexp^T directly (lhsT = v tile [j, d],
    rhs = Pexp^T [j, i] -> outT [d, i] accumulated over j-tiles in PSUM).
  - No max-subtraction in softmax (scores bounded); row sums come from an
    all-ones stationary matmul accumulated alongside PV, so normalization is
    one approx-reciprocal + one multiply.  outT overwrites qT storage.
  - Causal masking: only j-tiles with j <= chunk max are computed; the 4
    diagonal tiles per (head, i-chunk) get affine_select(fill=0) after exp.
  - wo matmul with lhsT = normalized outT slices, accumulated over the 4
    heads in PSUM; eviction is a plain copy, DMA'd to the partial output.

Scheduling notes (v5):
  - bf16 matmul operands: enables the fast-weight-load path (FWL is disabled
    for fp32), so LDWEIGHTS hides behind the previous matmul; halves DMA.
  - kT/qT live as per-chunk tiles so stage B's first scores depend only on
    chunk 0's rope, not the whole stage-A tail.
  - Two HWDGE issue queues: weights on the scalar-engine ring; x stream,
    rope tables and output stores on the sync-engine ring.  Weight loads are
    interleaved in PE-consumption order so the first matmul starts ~4 us in.
  - x streamed in 512 KB batches (4 k-tiles), quad-buffered.
  - RoPE eviction copy is the only read of the accumulation PSUM bank, and
    q-head ropes run before k, so the banks the next chunk's first matmuls
    need are released early.
  - v transposes for chunk sc are emitted mid-chunk sc+1, off the PE
    critical path.
  - Stage B pv/rs PSUM tiles double-buffered, st triple-buffered; softmax
    reciprocal is the fast approx DVE op.
  - All of wo is prefetched on both rings once the x stream has issued (dep
    edges keep it out of stage A's bandwidth); stage C has zero DMA waits,
    and its PSUM accumulators share the st tag (3 bufs).
"""

import math
from contextlib import ExitStack

import numpy as np

import concourse.bass as bass
import concourse.tile as tile
from concourse import bacc, mybir
from concourse.bass_utils import run_bass_kernel_spmd
from concourse.tile import add_dep_helper

# ---- problem shape (hardcoded per contract) ----
S = 2048           # seq len
D = 4096           # model dim
HD = 128           # head dim
N_CORES = 8
NQH = 4            # q heads per core
QCOLS = NQH * HD   # 512 wq columns per core
SC_N = 4           # seq chunks of 512
KT_N = D // 128    # 32 contraction tiles
JT_N = S // 128    # 16 key tiles
ECH_N = D // 512   # 8 output column chunks
XB = 4             # x k-tiles per DMA batch
XB_N = KT_N // XB  # 8 batches per seq chunk

F32 = mybir.dt.float32
F32R = mybir.dt.float32r
BF16 = mybir.dt.bfloat16

MM_DT = BF16          # matmul operand dtype: F32R (accurate) or BF16 (fast)

_BUILD_CACHE: dict = {}

STAGES = "ABC"  # debug knob: subset of stages to emit


def _emit_body(nc, tc, dram, out, causal: bool):
    with ExitStack() as ctx:
        def pool(name, bufs, space="SBUF"):
            return ctx.enter_context(tc.tile_pool(name=name, bufs=bufs, space=space))

        pools = {
            "persist": pool("persist", 1),
            "rope_tmp": pool("rope_tmp", 3),
        }
        persist = pools["persist"]

        # persistent activations, split per chunk so stage B's dependencies
        # are per-chunk, not whole-tensor
        kT_c = [
            persist.tile([128, 512], MM_DT, tag=f"kT{sc}", name=f"kT{sc}")
            for sc in range(SC_N)
        ]
        # qT doubles as outT: B(h, ic) consumes qT[h][ic] then writes the
        # normalized attention output into the same tile.
        qT_c = [
            [
                persist.tile([128, 512], MM_DT, tag=f"qT{h}_{sc}", name=f"qT{h}_{sc}")
                for sc in range(SC_N)
            ]
            for h in range(NQH)
        ]
        v_sb = persist.tile([128, JT_N, HD], MM_DT, tag="v", name="v_sb")
        ones_sb = persist.tile([128, 128], MM_DT, tag="ones", name="ones_sb")
        ident_sb = persist.tile([128, 128], MM_DT, tag="ident", name="ident_sb")
        rotm_sb = persist.tile([128, 128], MM_DT, tag="rotm", name="rotm_sb")

        last_xt_dma = None

        # ---- stage A: projections + RoPE, per 512-wide seq chunk ----
        with tc.tile_pool(name="w", bufs=1) as wpool, \
             tc.tile_pool(name="x", bufs=6) as xpool, \
             tc.tile_pool(name="rope_in", bufs=2) as rpool, \
             tc.tile_pool(name="vtf", bufs=1) as vtfpool, \
             tc.tile_pool(name="psA", bufs=1, space="PSUM") as psA:
            # resident weights, loaded in PE-consumption order: slices of 4
            # k-tiles for each of wq/wk/wv so the first matmuls start early
            wq_sb = wpool.tile([128, KT_N, QCOLS], MM_DT, tag="wq", name="wq_sb")
            wk_sb = wpool.tile([128, KT_N, HD], MM_DT, tag="wk", name="wk_sb")
            wv_sb = wpool.tile([128, KT_N, HD], MM_DT, tag="wv", name="wv_sb")
            for g in range(XB_N):
                gs = slice(g * XB, (g + 1) * XB)
                nc.scalar.dma_start(wq_sb[:, gs, :], dram["wq"][:, gs, :])
                nc.scalar.dma_start(wk_sb[:, gs, :], dram["wk"][:, gs, :])
                nc.scalar.dma_start(wv_sb[:, gs, :], dram["wv"][:, gs, :])
                if g == 0:
                    nc.scalar.dma_start(rotm_sb[:], dram["rotm"][:])
                if g == 1:
                    nc.scalar.dma_start(ident_sb[:], dram["ident"][:])
                    nc.scalar.dma_start(ones_sb[:], dram["ones"][:])

            vtf_all = vtfpool.tile(
                [128, SC_N, 512], MM_DT, tag="vtf", name="vtf_all"
            )

            def v_transposes(sc):
                for vi in range(4):
                    ptr = psA.tile(
                        [128, 128], MM_DT, tag="pstr", name="pstr", bufs=1
                    )
                    nc.tensor.transpose(
                        ptr[:],
                        vtf_all[:, sc, vi * 128:(vi + 1) * 128],
                        ident_sb[:],
                    )
                    nc.vector.tensor_copy(v_sb[:, sc * 4 + vi, :], ptr[:])

            def rope_apply(dst, ps, cos_t, sin_t):
                """dst = ps * cos + rot(ps) * sin; rot via PE perm matmul.

                The eviction copy is the only read of the accumulation bank,
                so it is released as soon as the copy completes."""
                qc = pools["rope_tmp"].tile(
                    [128, 512], MM_DT, tag="rope_qc", name="rqc"
                )
                nc.vector.tensor_copy(qc[:], ps)
                sw = psA.tile([128, 512], F32, tag="pssw", name="pssw", bufs=1)
                nc.tensor.matmul(sw[:], rotm_sb[:], qc[:], start=True, stop=True)
                tc_ = pools["rope_tmp"].tile(
                    [128, 512], F32, tag="rope_tc", name="rtc"
                )
                nc.vector.tensor_mul(tc_[:], qc[:], cos_t)
                qs = pools["rope_tmp"].tile(
                    [128, 512], F32, tag="rope_qs", name="rqs"
                )
                nc.vector.tensor_mul(qs[:], sw[:], sin_t)
                nc.vector.tensor_add(dst, tc_[:], qs[:])

            for sc in range(SC_N):
                rp = rpool.tile([HD, 4, 512], F32, tag="rp", name="rp")
                ps_q = [
                    psA.tile([128, 512], F32, tag=f"psq{h}", name=f"psq{h}")
                    for h in range(NQH)
                ]
                ps_k = psA.tile([128, 512], F32, tag="psk", name="psk")
                ps_vt = psA.tile([128, 512], F32, tag="psvt", name="psvt")
                for xb in range(XB_N):
                    # batched x stream: 4 contraction tiles per DMA
                    xt = xpool.tile([128, XB, 512], MM_DT, tag="xt", name="xt")
                    last_xt_dma = nc.sync.dma_start(xt[:], dram["xn"][sc, xb])
                    if xb == 1:
                        # rope table load sits behind the first x batch so it
                        # doesn't delay the very first matmuls
                        nc.sync.dma_start(rp[:], dram["rope"][sc])
                    if xb == 2 and sc > 0:
                        # previous chunk's v transposes, off the critical path
                        v_transposes(sc - 1)
                    for ki in range(XB):
                        kt = xb * XB + ki
                        first, last = kt == 0, kt == KT_N - 1
                        for h in range(NQH):
                            nc.tensor.matmul(
                                ps_q[h][:],
                                wq_sb[:, kt, h * 128:(h + 1) * 128],
                                xt[:, ki, :],
                                start=first,
                                stop=last,
                            )
                        nc.tensor.matmul(
                            ps_k[:], wk_sb[:, kt, :], xt[:, ki, :],
                            start=first, stop=last,
                        )
                        nc.tensor.matmul(
                            ps_vt[:], wv_sb[:, kt, :], xt[:, ki, :],
                            start=first, stop=last,
                        )

                cq, sq, ck, sk = rp[:, 0, :], rp[:, 1, :], rp[:, 2, :], rp[:, 3, :]
                # q ropes first: they release the PSUM banks the next chunk's
                # first matmuls write
                for h in range(NQH):
                    rope_apply(qT_c[h][sc][:], ps_q[h][:], cq, sq)
                rope_apply(kT_c[sc][:], ps_k[:], ck, sk)
                nc.vector.tensor_copy(vtf_all[:, sc, :], ps_vt[:])
            v_transposes(SC_N - 1)

        if "B" not in STAGES:
            # A-only debug: flush qT so the program has a live output
            with tc.tile_pool(name="dbgp", bufs=1) as dbgp:
                dbg = dbgp.tile([128, 2, 512], F32, tag="ob", name="dbg")
                nc.vector.tensor_copy(dbg[:, 0, :], qT_c[0][0][:])
                nc.sync.dma_start(out[0, 0], dbg[:])
            return
        # ---- stage B: attention (transposed scores), per (i-chunk, head) ----
        # wo prefetch into the SBUF freed by the stage-A weight pool (dep
        # edges keep the transfers behind the x stream); stage C then runs
        # with zero DMA waits.
        with tc.tile_pool(name="wo", bufs=1) as wopool, \
             tc.tile_pool(name="pexp", bufs=4) as pexpool, \
             tc.tile_pool(name="recip", bufs=2) as rcpool, \
             tc.tile_pool(name="outsb", bufs=4) as obpool, \
             tc.tile_pool(name="psB", bufs=1, space="PSUM") as psB:
            wo_sb = wopool.tile(
                [128, ECH_N, NQH, 512], MM_DT, tag="wo", name="wo_sb"
            )
            for ech in range(ECH_N):
                eng = nc.scalar if ech % 2 else nc.sync
                wdma = eng.dma_start(wo_sb[:, ech, :, :], dram["wo"][ech])
                if last_xt_dma is not None:
                    add_dep_helper(
                        wdma.ins, last_xt_dma.ins, sync=True,
                        reason="wo prefetch waits for x stream",
                    )

            for ic in range(SC_N):
                njt = 4 * (ic + 1) if causal else JT_N
                for h in range(NQH):
                    pv = psB.tile([128, 512], F32, tag="pspv", name="pspv", bufs=2)
                    rs = psB.tile([128, 512], F32, tag="psrs", name="psrs", bufs=2)
                    for jt in range(njt):
                        st = psB.tile(
                            [128, 512], F32, tag="psst", name="psst", bufs=3
                        )
                        nc.tensor.matmul(
                            st[:],
                            kT_c[jt // 4][:, (jt % 4) * 128:(jt % 4 + 1) * 128],
                            qT_c[h][ic][:],
                            start=True,
                            stop=True,
                        )
                        pe = pexpool.tile([128, 512], MM_DT, tag="pe", name="pe")
                        nc.scalar.activation(
                            pe[:], st[:], mybir.ActivationFunctionType.Exp
                        )
                        if causal and jt >= 4 * ic:
                            nc.gpsimd.affine_select(
                                out=pe[:],
                                in_=pe[:],
                                pattern=[[1, 512]],
                                compare_op=mybir.AluOpType.is_ge,
                                fill=0.0,
                                base=512 * ic - 128 * jt,
                                channel_multiplier=-1,
                            )
                        first, last = jt == 0, jt == njt - 1
                        nc.tensor.matmul(
                            pv[:], v_sb[:, jt, :], pe[:], start=first, stop=last
                        )
                        nc.tensor.matmul(
                            rs[:], ones_sb[:], pe[:], start=first, stop=last
                        )
                    rc = rcpool.tile([128, 512], F32, tag="rc", name="rc")
                    nc.vector.reciprocal_approx_fast(rc[:], rs[:])
                    nc.vector.tensor_mul(qT_c[h][ic][:], pv[:], rc[:])

            if "C" not in STAGES:
                dbg = obpool.tile([128, 2, 512], F32, tag="ob", name="dbg")
                nc.vector.tensor_copy(dbg[:, 0, :], qT_c[0][0][:])
                nc.sync.dma_start(out[0, 0], dbg[:])
                return
            # ---- stage C: wo matmul (outT lives in qT_c) ----
            for ech in range(ECH_N):
                for itp in range(JT_N // 2):
                    # pack 2 row-tiles per output DMA (halves DMA issue count)
                    ob = obpool.tile([128, 2, 512], F32, tag="ob", name="ob")
                    for ii in range(2):
                        it = itp * 2 + ii
                        # share the st tag: 3 PSUM bufs decouple eviction
                        # from the next accumulation chain
                        pc = psB.tile(
                            [128, 512], F32, tag="psst", name="psc", bufs=3
                        )
                        for h in range(NQH):
                            nc.tensor.matmul(
                                pc[:],
                                qT_c[h][it // 4][
                                    :, (it % 4) * 128:(it % 4 + 1) * 128
                                ],
                                wo_sb[:, ech, h, :],
                                start=h == 0,
                                stop=h == NQH - 1,
                            )
                        nc.vector.tensor_copy(ob[:, ii, :], pc[:])
                    # alternate HWDGE rings so completion latency overlaps
                    eng = nc.scalar if itp % 2 else nc.sync
                    eng.dma_start(out[ech, itp], ob[:])


def build_nc(causal: bool = True, reps: int = 1):
    nc = bacc.Bacc(
        "TRN2", target_bir_lowering=False, debug=False, num_devices=N_CORES
    )
    dram = {}
    for name, shape, dt in [
        # host-prepermuted layouts: every DMA reads/writes contiguous
        # per-partition runs
        ("xn", [SC_N, XB_N, 128, XB, 512], MM_DT),
        ("wq", [128, KT_N, QCOLS], MM_DT),
        ("wk", [128, KT_N, HD], MM_DT),
        ("wv", [128, KT_N, HD], MM_DT),
        ("wo", [ECH_N, 128, NQH, 512], MM_DT),
        ("rope", [SC_N, HD, 4, 512], F32),
        ("ones", [128, 128], MM_DT),
        ("ident", [128, 128], MM_DT),
        ("rotm", [128, 128], MM_DT),
    ]:
        dram[name] = nc.dram_tensor(name, shape, dt, kind="ExternalInput").ap()
    out = nc.dram_tensor("out", [ECH_N, JT_N // 2, 128, 2, 512], F32,
                         kind="ExternalOutput").ap()

    with tile.TileContext(nc) as tc:
        for _ in range(reps):
            _emit_body(nc, tc, dram, out, causal)

    nc.compile()
    return nc


def get_nc(causal: bool = True):
    if causal not in _BUILD_CACHE:
        _BUILD_CACHE[causal] = build_nc(causal)
    return _BUILD_CACHE[causal]


def _mm_np(a):
    return np.ascontiguousarray(a).astype(mybir.dt.np(MM_DT))


def prep_in_maps(x, sincos, wq, wk, wv, wo):
    """Host-side shard + layout prep. Returns list of per-core input dicts.

    All tensors are pre-permuted so that every device DMA moves contiguous
    per-partition runs (device DMA engines are far more efficient that way).
    """
    x = np.asarray(x, np.float32)
    assert x.shape == (1, S, D)
    # xn[sc, xb, p, ki, n] = x[sc*512 + n, (xb*XB + ki)*128 + p]
    xn = _mm_np(
        x[0].reshape(SC_N, 512, XB_N, XB, 128).transpose(0, 2, 4, 3, 1)
    )

    sincos = np.asarray(sincos, np.float32)
    sinT = np.ascontiguousarray(sincos[:S, :HD].T)
    cosT = np.ascontiguousarray(sincos[:S, HD:].T)
    scale = np.float32(1.0 / math.sqrt(HD))
    # rope[sc, d, tbl, n], tbl order: cosq, sinq, cosk, sink
    # (sign of rotate-half is folded into the rotm permutation matrix)
    rope = np.stack(
        [cosT * scale, sinT * scale, cosT, sinT], axis=0
    ).reshape(4, HD, SC_N, 512).transpose(2, 1, 0, 3)
    rope = np.ascontiguousarray(rope)

    # rot(t)[m] = -t[m+64] for m<64, +t[m-64] for m>=64, as lhsT[k, m]
    rotm = np.zeros((128, 128), np.float32)
    for m in range(64):
        rotm[m + 64, m] = -1.0
        rotm[m, m + 64] = 1.0

    wq = np.asarray(wq, np.float32)
    wk = np.asarray(wk, np.float32)
    wv = np.asarray(wv, np.float32)
    wo = np.asarray(wo, np.float32)

    in_maps = []
    for c in range(N_CORES):
        wq_c = wq[:, c * QCOLS:(c + 1) * QCOLS]          # [D, 512]
        wk_c = wk[:, c * HD:(c + 1) * HD]                # [D, 128]
        wv_c = wv[:, c * HD:(c + 1) * HD]
        wo_c = wo[c * QCOLS:(c + 1) * QCOLS, :]          # [512, D]
        in_maps.append(
            {
                "xn": xn,
                # wq[p, kt, m] = wq_c[kt*128 + p, m]
                "wq": _mm_np(
                    wq_c.reshape(KT_N, 128, QCOLS).transpose(1, 0, 2)
                ),
                "wk": _mm_np(
                    wk_c.reshape(KT_N, 128, HD).transpose(1, 0, 2)
                ),
                "wv": _mm_np(
                    wv_c.reshape(KT_N, 128, HD).transpose(1, 0, 2)
                ),
                # wo[ech, p, a, n] = wo_c[a*128 + p, ech*512 + n]
                "wo": _mm_np(
                    wo_c.reshape(NQH, 128, ECH_N, 512).transpose(2, 1, 0, 3)
                ),
                "rope": rope,
                "ones": _mm_np(np.ones((128, 128), np.float32)),
                "ident": _mm_np(np.eye(128, dtype=np.float32)),
                "rotm": _mm_np(rotm),
            }
        )
    return in_maps


def unpermute_out(out_n):
    """out_n [ech, itp, p, ii, n] -> out [S, D]."""
    return np.ascontiguousarray(
        out_n.transpose(1, 3, 2, 0, 4).reshape(S, D)
    )


def check_mask(full_causal_mask, start_pos) -> bool:
    """Returns True for causal (tril) mask, False for all-allowed."""
    sp = int(start_pos)
    assert sp == 0, f"kernel specialized for start_pos=0, got {sp}"
    m = np.asarray(full_causal_mask)
    assert m.shape == (1, 1, S, S)
    m = m[0, 0]
    tril = np.tril(np.ones((S, S), dtype=bool))
    if (m == tril).all():
        return True
    if m.all():
        return False
    raise AssertionError("unsupported mask pattern")


def kernel(
    x,
    start_pos,
    sincos,
    full_causal_mask,
    wq,
    wk,
    wv,
    wo,
    cache_k,
    cache_v,
):
    causal = check_mask(full_causal_mask, start_pos)
    # cache_k/cache_v are zero and fully overwritten in the attended region
    # (start_pos=0, seq_len == max_seq_len) — they do not affect the output.
    nc = get_nc(causal)
    in_maps = prep_in_maps(x, sincos, wq, wk, wv, wo)
    res = run_bass_kernel_spmd(nc, in_maps, list(range(N_CORES)))
    acc = res.results[0]["out"].astype(np.float32)
    for c in range(1, N_CORES):
        acc = acc + res.results[c]["out"]
    return unpermute_out(acc)[np.newaxis]


# revision 11
# speedup vs baseline: 1.0106x; 1.0106x over previous
"""Tensor-parallel GQA attention (Llama-3-8B shape, prefill, start_pos=0) on 8
Trainium2 NeuronCores.

Sharding: core i owns kv-head i and q-heads 4i..4i+3 — wq/wk/wv column-shards,
wo row-shard, x replicated.  Each core computes a partial [2048, 4096] output
(its heads pushed through its wo rows); the host sums the 8 partials
(all-reduce equivalent).

Per-core kernel layout (matmuls N=512, bf16 operands, fp32 PSUM accumulate):
  - xT [D, S] on device; projections computed with weights as the stationary
    operand, giving qT/kT/vT in [head_dim, seq] layout directly.
  - RoPE in [head_dim, seq] layout; rotate-half is a PE matmul against a
    signed permutation matrix (keeps the DVE FIFO free of DMA waits), the
    1/sqrt(hd) score scale is folded into q's sin/cos tables.
  - Scores computed transposed, ST[j, i] = kT.T @ qT, so exp needs no
    transposes and PV consumes Pexp^T directly (lhsT = v tile [j, d],
    rhs = Pexp^T [j, i] -> outT [d, i] accumulated over j-tiles in PSUM).
  - No max-subtraction in softmax (scores bounded); row sums come from an
    all-ones stationary matmul accumulated alongside PV, so normalization is
    one approx-reciprocal + one multiply.  outT overwrites qT storage.
  - Causal masking: only j-tiles with j <= chunk max are computed; the 4
    diagonal tiles per (head, i-chunk) get affine_select(fill=0) after exp.
  - wo matmul with lhsT = normalized outT slices, accumulated over the 4
    heads in PSUM; eviction is a plain copy, DMA'd to the partial output.

Scheduling notes (v5):
  - bf16 matmul operands: enables the fast-weight-load path (FWL is disabled
    for fp32), so LDWEIGHTS hides behind the previous matmul; halves DMA.
  - kT/qT live as per-chunk tiles so stage B's first scores depend only on
    chunk 0's rope, not the whole stage-A tail.
  - Two HWDGE issue queues: weights on the scalar-engine ring; x stream,
    rope tables and output stores on the sync-engine ring.  Weight loads are
    interleaved in PE-consumption order so the first matmul starts ~4 us in.
  - x streamed in 512 KB batches (4 k-tiles), quad-buffered.
  - RoPE eviction copy is the only read of the accumulation PSUM bank, and
    q-head ropes run before k, so the banks the next chunk's first matmuls
    need are released early.
  - v transposes for chunk sc are emitted mid-chunk sc+1, off the PE
    critical path.
  - Stage B pv/rs PSUM tiles double-buffered, st triple-buffered; softmax
    reciprocal is the fast approx DVE op.
  - All of wo is prefetched on both rings once the x stream has issued (dep
    edges keep it out of stage A's bandwidth); stage C has zero DMA waits,
    and its PSUM accumulators share the st tag (3 bufs).
"""

import math
from contextlib import ExitStack

import numpy as np

import concourse.bass as bass
import concourse.tile as tile
from concourse import bacc, mybir
from concourse.bass_utils import run_bass_kernel_spmd
from concourse.tile import add_dep_helper

# ---- problem shape (hardcoded per contract) ----
S = 2048           # seq len
D = 4096           # model dim
HD = 128           # head dim
N_CORES = 8
NQH = 4            # q heads per core
QCOLS = NQH * HD   # 512 wq columns per core
SC_N = 4           # seq chunks of 512
KT_N = D // 128    # 32 contraction tiles
JT_N = S // 128    # 16 key tiles
ECH_N = D // 512   # 8 output column chunks
XB = 4             # x k-tiles per DMA batch
XB_N = KT_N // XB  # 8 batches per seq chunk

F32 = mybir.dt.float32
F32R = mybir.dt.float32r
BF16 = mybir.dt.bfloat16

MM_DT = BF16          # matmul operand dtype: F32R (accurate) or BF16 (fast)

_BUILD_CACHE: dict = {}

STAGES = "ABC"  # debug knob: subset of stages to emit


def _emit_body(nc, tc, dram, out, causal: bool):
    with ExitStack() as ctx:
        def pool(name, bufs, space="SBUF"):
            return ctx.enter_context(tc.tile_pool(name=name, bufs=bufs, space=space))

        pools = {
            "persist": pool("persist", 1),
            "rope_tmp": pool("rope_tmp", 3),
        }
        persist = pools["persist"]

        # persistent activations, split per chunk so stage B's dependencies
        # are per-chunk, not whole-tensor
        kT_c = [
            persist.tile([128, 512], MM_DT, tag=f"kT{sc}", name=f"kT{sc}")
            for sc in range(SC_N)
        ]
        # qT doubles as outT: B(h, ic) consumes qT[h][ic] then writes the
        # normalized attention output into the same tile.
        qT_c = [
            [
                persist.tile([128, 512], MM_DT, tag=f"qT{h}_{sc}", name=f"qT{h}_{sc}")
                for sc in range(SC_N)
            ]
            for h in range(NQH)
        ]
        v_sb = persist.tile([128, JT_N, HD], MM_DT, tag="v", name="v_sb")
        ones_sb = persist.tile([128, 128], MM_DT, tag="ones", name="ones_sb")
        ident_sb = persist.tile([128, 128], MM_DT, tag="ident", name="ident_sb")
        rotm_sb = persist.tile([128, 128], MM_DT, tag="rotm", name="rotm_sb")

        last_xt_dma = None

        # ---- stage A: projections + RoPE, per 512-wide seq chunk ----
        with tc.tile_pool(name="w", bufs=1) as wpool, \
             tc.tile_pool(name="x", bufs=6) as xpool, \
             tc.tile_pool(name="rope_in", bufs=2) as rpool, \
             tc.tile_pool(name="vtf", bufs=1) as vtfpool, \
             tc.tile_pool(name="psA", bufs=1, space="PSUM") as psA:
            # resident weights, loaded in PE-consumption order: slices of 4
            # k-tiles for each of wq/wk/wv so the first matmuls start early
            wq_sb = wpool.tile([128, KT_N, QCOLS], MM_DT, tag="wq", name="wq_sb")
            wk_sb = wpool.tile([128, KT_N, HD], MM_DT, tag="wk", name="wk_sb")
            wv_sb = wpool.tile([128, KT_N, HD], MM_DT, tag="wv", name="wv_sb")
            wslices = [slice(0, 1), slice(1, XB)] + [
                slice(g * XB, (g + 1) * XB) for g in range(1, XB_N)
            ]
            for g, gs in enumerate(wslices):
                nc.scalar.dma_start(wq_sb[:, gs, :], dram["wq"][:, gs, :])
                nc.scalar.dma_start(wk_sb[:, gs, :], dram["wk"][:, gs, :])
                nc.scalar.dma_start(wv_sb[:, gs, :], dram["wv"][:, gs, :])
                if g == 0:
                    nc.scalar.dma_start(rotm_sb[:], dram["rotm"][:])
                if g == 2:
                    nc.scalar.dma_start(ident_sb[:], dram["ident"][:])
                    nc.scalar.dma_start(ones_sb[:], dram["ones"][:])

            vtf_all = vtfpool.tile(
                [128, SC_N, 512], MM_DT, tag="vtf", name="vtf_all"
            )

            def v_transposes(sc):
                for vi in range(4):
                    ptr = psA.tile(
                        [128, 128], MM_DT, tag="pstr", name="pstr", bufs=1
                    )
                    nc.tensor.transpose(
                        ptr[:],
                        vtf_all[:, sc, vi * 128:(vi + 1) * 128],
                        ident_sb[:],
                    )
                    nc.vector.tensor_copy(v_sb[:, sc * 4 + vi, :], ptr[:])

            def rope_apply(dst, ps, cos_t, sin_t, dma_swap=False):
                """dst = ps * cos + rot(ps) * sin_signed.

                rot is a PE matmul against a plain permutation (keeps the DVE
                FIFO free of DMA waits); with dma_swap=True it is two
                SBUF->SBUF DMAs instead, keeping the PE stream free — used
                for the last chunk so stage B's scores aren't stuck behind
                rope matmuls nothing depends on yet.  The eviction copy is
                the only read of the accumulation bank, so it is released as
                soon as the copy completes."""
                qc = pools["rope_tmp"].tile(
                    [128, 512], MM_DT, tag="rope_qc", name="rqc"
                )
                nc.vector.tensor_copy(qc[:], ps)
                tc_ = pools["rope_tmp"].tile(
                    [128, 512], F32, tag="rope_tc", name="rtc"
                )
                qs = pools["rope_tmp"].tile(
                    [128, 512], F32, tag="rope_qs", name="rqs"
                )
                if dma_swap:
                    qsw = pools["rope_tmp"].tile(
                        [128, 512], MM_DT, tag="rope_qsw", name="rqsw"
                    )
                    nc.scalar.dma_start(qsw[0:64, :], qc[64:128, :])
                    nc.scalar.dma_start(qsw[64:128, :], qc[0:64, :])
                    nc.vector.tensor_mul(tc_[:], qc[:], cos_t)
                    nc.vector.tensor_mul(qs[:], qsw[:], sin_t)
                else:
                    sw = psA.tile(
                        [128, 512], F32, tag="pssw", name="pssw", bufs=1
                    )
                    nc.tensor.matmul(
                        sw[:], rotm_sb[:], qc[:], start=True, stop=True
                    )
                    nc.vector.tensor_mul(tc_[:], qc[:], cos_t)
                    nc.vector.tensor_mul(qs[:], sw[:], sin_t)
                nc.vector.tensor_add(dst, tc_[:], qs[:])

            for sc in range(SC_N):
                rp = rpool.tile([HD, 4, 512], F32, tag="rp", name="rp")
                ps_q = [
                    psA.tile([128, 512], F32, tag=f"psq{h}", name=f"psq{h}")
                    for h in range(NQH)
                ]
                ps_k = psA.tile([128, 512], F32, tag="psk", name="psk")
                ps_vt = psA.tile([128, 512], F32, tag="psvt", name="psvt")
                for xb in range(XB_N):
                    # batched x stream: 4 contraction tiles per DMA
                    xt = xpool.tile([128, XB, 512], MM_DT, tag="xt", name="xt")
                    last_xt_dma = nc.sync.dma_start(xt[:], dram["xn"][sc, xb])
                    if xb == 1:
                        # rope table load sits behind the first x batch so it
                        # doesn't delay the very first matmuls
                        nc.sync.dma_start(rp[:], dram["rope"][sc])
                    if xb == 2 and sc > 0:
                        # previous chunk's v transposes, off the critical path
                        v_transposes(sc - 1)
                    for ki in range(XB):
                        kt = xb * XB + ki
                        first, last = kt == 0, kt == KT_N - 1
                        for h in range(NQH):
                            nc.tensor.matmul(
                                ps_q[h][:],
                                wq_sb[:, kt, h * 128:(h + 1) * 128],
                                xt[:, ki, :],
                                start=first,
                                stop=last,
                            )
                        nc.tensor.matmul(
                            ps_k[:], wk_sb[:, kt, :], xt[:, ki, :],
                            start=first, stop=last,
                        )
                        nc.tensor.matmul(
                            ps_vt[:], wv_sb[:, kt, :], xt[:, ki, :],
                            start=first, stop=last,
                        )

                cq, sq, ck, sk = rp[:, 0, :], rp[:, 1, :], rp[:, 2, :], rp[:, 3, :]
                last = sc == SC_N - 1
                if last:
                    # vtf copy + final v transposes first, so the PE stream
                    # runs straight into stage B; the DMA-swap rope keeps the
                    # rope entirely off the PE stream (nothing in early B
                    # depends on chunk 3's rope)
                    nc.vector.tensor_copy(vtf_all[:, sc, :], ps_vt[:])
                    v_transposes(sc)
                # q ropes first: they release the PSUM banks the next chunk's
                # first matmuls write
                for h in range(NQH):
                    rope_apply(qT_c[h][sc][:], ps_q[h][:], cq, sq, dma_swap=last)
                rope_apply(kT_c[sc][:], ps_k[:], ck, sk, dma_swap=last)
                if not last:
                    nc.vector.tensor_copy(vtf_all[:, sc, :], ps_vt[:])

        if "B" not in STAGES:
            # A-only debug: flush qT so the program has a live output
            with tc.tile_pool(name="dbgp", bufs=1) as dbgp:
                dbg = dbgp.tile([128, 2, 512], F32, tag="ob", name="dbg")
                nc.vector.tensor_copy(dbg[:, 0, :], qT_c[0][0][:])
                nc.sync.dma_start(out[0, 0], dbg[:])
            return
        # ---- stage B: attention (transposed scores), per (i-chunk, head) ----
        # wo prefetch into the SBUF freed by the stage-A weight pool (dep
        # edges keep the transfers behind the x stream); stage C then runs
        # with zero DMA waits.
        with tc.tile_pool(name="wo", bufs=1) as wopool, \
             tc.tile_pool(name="pexp", bufs=6) as pexpool, \
             tc.tile_pool(name="recip", bufs=2) as rcpool, \
             tc.tile_pool(name="outsb", bufs=4) as obpool, \
             tc.tile_pool(name="psB", bufs=1, space="PSUM") as psB:
            wo_sb = wopool.tile(
                [128, ECH_N, NQH, 512], MM_DT, tag="wo", name="wo_sb"
            )
            for ech in range(ECH_N):
                eng = nc.scalar if ech % 2 else nc.sync
                wdma = eng.dma_start(wo_sb[:, ech, :, :], dram["wo"][ech])
                if last_xt_dma is not None:
                    add_dep_helper(
                        wdma.ins, last_xt_dma.ins, sync=True,
                        reason="wo prefetch waits for x stream",
                    )

            for ic in range(SC_N):
                njt = 4 * (ic + 1) if causal else JT_N
                for h in range(NQH):
                    pv = psB.tile([128, 512], F32, tag="pspv", name="pspv", bufs=2)
                    rs = psB.tile([128, 512], F32, tag="psrs", name="psrs", bufs=2)
                    for jt in range(njt):
                        st = psB.tile(
                            [128, 512], F32, tag="psst", name="psst", bufs=4
                        )
                        nc.tensor.matmul(
                            st[:],
                            kT_c[jt // 4][:, (jt % 4) * 128:(jt % 4 + 1) * 128],
                            qT_c[h][ic][:],
                            start=True,
                            stop=True,
                        )
                        pe = pexpool.tile([128, 512], MM_DT, tag="pe", name="pe")
                        nc.scalar.activation(
                            pe[:], st[:], mybir.ActivationFunctionType.Exp
                        )
                        if causal and jt >= 4 * ic:
                            nc.gpsimd.affine_select(
                                out=pe[:],
                                in_=pe[:],
                                pattern=[[1, 512]],
                                compare_op=mybir.AluOpType.is_ge,
                                fill=0.0,
                                base=512 * ic - 128 * jt,
                                channel_multiplier=-1,
                            )
                        first, last = jt == 0, jt == njt - 1
                        nc.tensor.matmul(
                            pv[:], v_sb[:, jt, :], pe[:], start=first, stop=last
                        )
                        nc.tensor.matmul(
                            rs[:], ones_sb[:], pe[:], start=first, stop=last
                        )
                    rc = rcpool.tile([128, 512], F32, tag="rc", name="rc")
                    nc.vector.reciprocal_approx_fast(rc[:], rs[:])
                    nc.vector.tensor_mul(qT_c[h][ic][:], pv[:], rc[:])

            if "C" not in STAGES:
                dbg = obpool.tile([128, 2, 512], F32, tag="ob", name="dbg")
                nc.vector.tensor_copy(dbg[:, 0, :], qT_c[0][0][:])
                nc.sync.dma_start(out[0, 0], dbg[:])
                return
            # ---- stage C: wo matmul (outT lives in qT_c) ----
            for ech in range(ECH_N):
                for itp in range(JT_N // 2):
                    # pack 2 row-tiles per output DMA (halves DMA issue count)
                    ob = obpool.tile([128, 2, 512], F32, tag="ob", name="ob")
                    for ii in range(2):
                        it = itp * 2 + ii
                        # share the st tag: 3 PSUM bufs decouple eviction
                        # from the next accumulation chain
                        pc = psB.tile(
                            [128, 512], F32, tag="psst", name="psc", bufs=4
                        )
                        for h in range(NQH):
                            nc.tensor.matmul(
                                pc[:],
                                qT_c[h][it // 4][
                                    :, (it % 4) * 128:(it % 4 + 1) * 128
                                ],
                                wo_sb[:, ech, h, :],
                                start=h == 0,
                                stop=h == NQH - 1,
                            )
                        nc.vector.tensor_copy(ob[:, ii, :], pc[:])
                    # alternate HWDGE rings so completion latency overlaps
                    eng = nc.scalar if itp % 2 else nc.sync
                    eng.dma_start(out[ech, itp], ob[:])


def build_nc(causal: bool = True, reps: int = 1):
    nc = bacc.Bacc(
        "TRN2", target_bir_lowering=False, debug=False, num_devices=N_CORES
    )
    dram = {}
    for name, shape, dt in [
        # host-prepermuted layouts: every DMA reads/writes contiguous
        # per-partition runs
        ("xn", [SC_N, XB_N, 128, XB, 512], MM_DT),
        ("wq", [128, KT_N, QCOLS], MM_DT),
        ("wk", [128, KT_N, HD], MM_DT),
        ("wv", [128, KT_N, HD], MM_DT),
        ("wo", [ECH_N, 128, NQH, 512], MM_DT),
        ("rope", [SC_N, HD, 4, 512], F32),
        ("ones", [128, 128], MM_DT),
        ("ident", [128, 128], MM_DT),
        ("rotm", [128, 128], MM_DT),
    ]:
        dram[name] = nc.dram_tensor(name, shape, dt, kind="ExternalInput").ap()
    out = nc.dram_tensor("out", [ECH_N, JT_N // 2, 128, 2, 512], F32,
                         kind="ExternalOutput").ap()

    with tile.TileContext(nc) as tc:
        for _ in range(reps):
            _emit_body(nc, tc, dram, out, causal)

    nc.compile()
    return nc


def get_nc(causal: bool = True):
    if causal not in _BUILD_CACHE:
        _BUILD_CACHE[causal] = build_nc(causal)
    return _BUILD_CACHE[causal]


def _mm_np(a):
    return np.ascontiguousarray(a).astype(mybir.dt.np(MM_DT))


def prep_in_maps(x, sincos, wq, wk, wv, wo):
    """Host-side shard + layout prep. Returns list of per-core input dicts.

    All tensors are pre-permuted so that every device DMA moves contiguous
    per-partition runs (device DMA engines are far more efficient that way).
    """
    x = np.asarray(x, np.float32)
    assert x.shape == (1, S, D)
    # xn[sc, xb, p, ki, n] = x[sc*512 + n, (xb*XB + ki)*128 + p]
    xn = _mm_np(
        x[0].reshape(SC_N, 512, XB_N, XB, 128).transpose(0, 2, 4, 3, 1)
    )

    sincos = np.asarray(sincos, np.float32)
    sinT = np.ascontiguousarray(sincos[:S, :HD].T)
    cosT = np.ascontiguousarray(sincos[:S, HD:].T)
    scale = np.float32(1.0 / math.sqrt(HD))
    # rope[sc, d, tbl, n], tbl order: cosq, sinq, cosk, sink; sin tables
    # carry the rotate-half sign (rows 0:64 negated), rotm is a plain perm
    sin_sgn = sinT.copy()
    sin_sgn[:64] = -sinT[:64]
    rope = np.stack(
        [cosT * scale, sin_sgn * scale, cosT, sin_sgn], axis=0
    ).reshape(4, HD, SC_N, 512).transpose(2, 1, 0, 3)
    rope = np.ascontiguousarray(rope)

    # rot(t)[m] = t[(m+64) % 128] as lhsT[k, m]; the sign lives in the
    # sin tables so the DMA-swap and PE-matmul rope variants match
    rotm = np.zeros((128, 128), np.float32)
    for m in range(64):
        rotm[m + 64, m] = 1.0
        rotm[m, m + 64] = 1.0

    wq = np.asarray(wq, np.float32)
    wk = np.asarray(wk, np.float32)
    wv = np.asarray(wv, np.float32)
    wo = np.asarray(wo, np.float32)

    in_maps = []
    for c in range(N_CORES):
        wq_c = wq[:, c * QCOLS:(c + 1) * QCOLS]          # [D, 512]
        wk_c = wk[:, c * HD:(c + 1) * HD]                # [D, 128]
        wv_c = wv[:, c * HD:(c + 1) * HD]
        wo_c = wo[c * QCOLS:(c + 1) * QCOLS, :]          # [512, D]
        in_maps.append(
            {
                "xn": xn,
                # wq[p, kt, m] = wq_c[kt*128 + p, m]
                "wq": _mm_np(
                    wq_c.reshape(KT_N, 128, QCOLS).transpose(1, 0, 2)
                ),
                "wk": _mm_np(
                    wk_c.reshape(KT_N, 128, HD).transpose(1, 0, 2)
                ),
                "wv": _mm_np(
                    wv_c.reshape(KT_N, 128, HD).transpose(1, 0, 2)
                ),
                # wo[ech, p, a, n] = wo_c[a*128 + p, ech*512 + n]
                "wo": _mm_np(
                    wo_c.reshape(NQH, 128, ECH_N, 512).transpose(2, 1, 0, 3)
                ),
                "rope": rope,
                "ones": _mm_np(np.ones((128, 128), np.float32)),
                "ident": _mm_np(np.eye(128, dtype=np.float32)),
                "rotm": _mm_np(rotm),
            }
        )
    return in_maps


def unpermute_out(out_n):
    """out_n [ech, itp, p, ii, n] -> out [S, D]."""
    return np.ascontiguousarray(
        out_n.transpose(1, 3, 2, 0, 4).reshape(S, D)
    )


def check_mask(full_causal_mask, start_pos) -> bool:
    """Returns True for causal (tril) mask, False for all-allowed."""
    sp = int(start_pos)
    assert sp == 0, f"kernel specialized for start_pos=0, got {sp}"
    m = np.asarray(full_causal_mask)
    assert m.shape == (1, 1, S, S)
    m = m[0, 0]
    tril = np.tril(np.ones((S, S), dtype=bool))
    if (m == tril).all():
        return True
    if m.all():
        return False
    raise AssertionError("unsupported mask pattern")


def kernel(
    x,
    start_pos,
    sincos,
    full_causal_mask,
    wq,
    wk,
    wv,
    wo,
    cache_k,
    cache_v,
):
    causal = check_mask(full_causal_mask, start_pos)
    # cache_k/cache_v are zero and fully overwritten in the attended region
    # (start_pos=0, seq_len == max_seq_len) — they do not affect the output.
    nc = get_nc(causal)
    in_maps = prep_in_maps(x, sincos, wq, wk, wv, wo)
    res = run_bass_kernel_spmd(nc, in_maps, list(range(N_CORES)))
    acc = res.results[0]["out"].astype(np.float32)
    for c in range(1, N_CORES):
        acc = acc + res.results[c]["out"]
    return unpermute_out(acc)[np.newaxis]


# revision 14
# speedup vs baseline: 1.0408x; 1.0299x over previous
"""Tensor-parallel GQA attention (Llama-3-8B shape, prefill, start_pos=0) on 8
Trainium2 NeuronCores.

Sharding: core i owns kv-head i and q-heads 4i..4i+3 — wq/wk/wv column-shards,
wo row-shard, x replicated.  Each core computes a partial [2048, 4096] output
(its heads pushed through its wo rows); the host sums the 8 partials
(all-reduce equivalent).

Per-core kernel layout (matmuls N=512, bf16 operands, fp32 PSUM accumulate):
  - xT [D, S] on device; projections computed with weights as the stationary
    operand, giving qT/kT/vT in [head_dim, seq] layout directly.
  - RoPE in [head_dim, seq] layout; rotate-half is a PE matmul against a
    signed permutation matrix (keeps the DVE FIFO free of DMA waits), the
    1/sqrt(hd) score scale is folded into q's sin/cos tables.
  - Scores computed transposed, ST[j, i] = kT.T @ qT, so exp needs no
    transposes and PV consumes Pexp^T directly (lhsT = v tile [j, d],
    rhs = Pexp^T [j, i] -> outT [d, i] accumulated over j-tiles in PSUM).
  - No max-subtraction in softmax (scores bounded); row sums come from an
    all-ones stationary matmul accumulated alongside PV, so normalization is
    one approx-reciprocal + one multiply.  outT overwrites qT storage.
  - Causal masking: only j-tiles with j <= chunk max are computed; the 4
    diagonal tiles per (head, i-chunk) get affine_select(fill=0) after exp.
  - wo matmul with lhsT = normalized outT slices, accumulated over the 4
    heads in PSUM; eviction is a plain copy, DMA'd to the partial output.

Scheduling notes (v5):
  - bf16 matmul operands: enables the fast-weight-load path (FWL is disabled
    for fp32), so LDWEIGHTS hides behind the previous matmul; halves DMA.
  - kT/qT live as per-chunk tiles so stage B's first scores depend only on
    chunk 0's rope, not the whole stage-A tail.
  - Two HWDGE issue queues: weights on the scalar-engine ring; x stream,
    rope tables and output stores on the sync-engine ring.  Weight loads are
    interleaved in PE-consumption order so the first matmul starts ~4 us in.
  - x streamed in 512 KB batches (4 k-tiles), quad-buffered.
  - RoPE eviction copy is the only read of the accumulation PSUM bank, and
    q-head ropes run before k, so the banks the next chunk's first matmuls
    need are released early.
  - v transposes for chunk sc are emitted mid-chunk sc+1, off the PE
    critical path.
  - Stage B pv/rs PSUM tiles double-buffered, st triple-buffered; softmax
    reciprocal is the fast approx DVE op.
  - All of wo is prefetched on both rings once the x stream has issued (dep
    edges keep it out of stage A's bandwidth); stage C has zero DMA waits,
    and its PSUM accumulators share the st tag (3 bufs).
"""

import math
from contextlib import ExitStack

import numpy as np

import concourse.bass as bass
import concourse.tile as tile
from concourse import bacc, mybir
from concourse.bass_utils import run_bass_kernel_spmd
from concourse.tile import add_dep_helper

# ---- problem shape (hardcoded per contract) ----
S = 2048           # seq len
D = 4096           # model dim
HD = 128           # head dim
N_CORES = 8
NQH = 4            # q heads per core
QCOLS = NQH * HD   # 512 wq columns per core
SC_N = 4           # seq chunks of 512
KT_N = D // 128    # 32 contraction tiles
JT_N = S // 128    # 16 key tiles
ECH_N = D // 512   # 8 output column chunks
XB = 4             # x k-tiles per DMA batch
XB_N = KT_N // XB  # 8 batches per seq chunk

F32 = mybir.dt.float32
F32R = mybir.dt.float32r
BF16 = mybir.dt.bfloat16

MM_DT = BF16          # matmul operand dtype: F32R (accurate) or BF16 (fast)

_BUILD_CACHE: dict = {}

STAGES = "ABC"  # debug knob: subset of stages to emit


def _emit_body(nc, tc, dram, out, causal: bool):
    with ExitStack() as ctx:
        def pool(name, bufs, space="SBUF"):
            return ctx.enter_context(tc.tile_pool(name=name, bufs=bufs, space=space))

        pools = {
            "persist": pool("persist", 1),
            "rope_tmp": pool("rope_tmp", 3),
        }
        persist = pools["persist"]

        # persistent activations, split per chunk so stage B's dependencies
        # are per-chunk, not whole-tensor
        kT_c = [
            persist.tile([128, 512], MM_DT, tag=f"kT{sc}", name=f"kT{sc}")
            for sc in range(SC_N)
        ]
        # qT doubles as outT: B(h, ic) consumes qT[h][ic] then writes the
        # normalized attention output into the same tile.
        qT_c = [
            [
                persist.tile([128, 512], MM_DT, tag=f"qT{h}_{sc}", name=f"qT{h}_{sc}")
                for sc in range(SC_N)
            ]
            for h in range(NQH)
        ]
        v_sb = persist.tile([128, JT_N, HD], MM_DT, tag="v", name="v_sb")
        ones_sb = persist.tile([128, 128], MM_DT, tag="ones", name="ones_sb")
        ident_sb = persist.tile([128, 128], MM_DT, tag="ident", name="ident_sb")
        rotm_sb = persist.tile([128, 128], MM_DT, tag="rotm", name="rotm_sb")

        last_xt_dma = None

        # ---- stage A: projections + RoPE, per 512-wide seq chunk ----
        with tc.tile_pool(name="w", bufs=1) as wpool, \
             tc.tile_pool(name="x", bufs=6) as xpool, \
             tc.tile_pool(name="rope_in", bufs=2) as rpool, \
             tc.tile_pool(name="vtf", bufs=1) as vtfpool, \
             tc.tile_pool(name="psA", bufs=1, space="PSUM") as psA:
            # resident weights, loaded in PE-consumption order: slices of 4
            # k-tiles for each of wq/wk/wv so the first matmuls start early
            wq_sb = wpool.tile([128, KT_N, QCOLS], MM_DT, tag="wq", name="wq_sb")
            wk_sb = wpool.tile([128, KT_N, HD], MM_DT, tag="wk", name="wk_sb")
            wv_sb = wpool.tile([128, KT_N, HD], MM_DT, tag="wv", name="wv_sb")
            wslices = [slice(0, 1), slice(1, XB)] + [
                slice(g * XB, (g + 1) * XB) for g in range(1, XB_N)
            ]
            for g, gs in enumerate(wslices):
                nc.scalar.dma_start(wq_sb[:, gs, :], dram["wq"][:, gs, :])
                nc.scalar.dma_start(wk_sb[:, gs, :], dram["wk"][:, gs, :])
                nc.scalar.dma_start(wv_sb[:, gs, :], dram["wv"][:, gs, :])
                if g == 0:
                    nc.scalar.dma_start(rotm_sb[:], dram["rotm"][:])
                if g == 2:
                    nc.scalar.dma_start(ident_sb[:], dram["ident"][:])
                    nc.scalar.dma_start(ones_sb[:], dram["ones"][:])

            vtf_all = vtfpool.tile(
                [128, SC_N, 512], MM_DT, tag="vtf", name="vtf_all"
            )

            def v_transposes(sc):
                for vi in range(4):
                    ptr = psA.tile(
                        [128, 128], MM_DT, tag="pstr", name="pstr", bufs=1
                    )
                    nc.tensor.transpose(
                        ptr[:],
                        vtf_all[:, sc, vi * 128:(vi + 1) * 128],
                        ident_sb[:],
                    )
                    nc.vector.tensor_copy(v_sb[:, sc * 4 + vi, :], ptr[:])

            def rope_apply(dst, ps, cos_t, sin_t, dma_swap=False):
                """dst = ps * cos + rot(ps) * sin_signed.

                rot is a PE matmul against a plain permutation (keeps the DVE
                FIFO free of DMA waits); with dma_swap=True it is two
                SBUF->SBUF DMAs instead, keeping the PE stream free — used
                for the last chunk so stage B's scores aren't stuck behind
                rope matmuls nothing depends on yet.  The eviction copy is
                the only read of the accumulation bank, so it is released as
                soon as the copy completes."""
                qc = pools["rope_tmp"].tile(
                    [128, 512], MM_DT, tag="rope_qc", name="rqc"
                )
                nc.vector.tensor_copy(qc[:], ps)
                tc_ = pools["rope_tmp"].tile(
                    [128, 512], F32, tag="rope_tc", name="rtc"
                )
                qs = pools["rope_tmp"].tile(
                    [128, 512], F32, tag="rope_qs", name="rqs"
                )
                if dma_swap:
                    qsw = pools["rope_tmp"].tile(
                        [128, 512], MM_DT, tag="rope_qsw", name="rqsw"
                    )
                    nc.scalar.dma_start(qsw[0:64, :], qc[64:128, :])
                    nc.scalar.dma_start(qsw[64:128, :], qc[0:64, :])
                    nc.vector.tensor_mul(tc_[:], qc[:], cos_t)
                    nc.vector.tensor_mul(qs[:], qsw[:], sin_t)
                else:
                    sw = psA.tile(
                        [128, 512], F32, tag="pssw", name="pssw", bufs=1
                    )
                    nc.tensor.matmul(
                        sw[:], rotm_sb[:], qc[:], start=True, stop=True
                    )
                    nc.vector.tensor_mul(tc_[:], qc[:], cos_t)
                    nc.vector.tensor_mul(qs[:], sw[:], sin_t)
                nc.vector.tensor_add(dst, tc_[:], qs[:])

            for sc in range(SC_N):
                rp = rpool.tile([HD, 4, 512], F32, tag="rp", name="rp")
                ps_q = [
                    psA.tile([128, 512], F32, tag=f"psq{h}", name=f"psq{h}")
                    for h in range(NQH)
                ]
                ps_k = psA.tile([128, 512], F32, tag="psk", name="psk")
                ps_vt = psA.tile([128, 512], F32, tag="psvt", name="psvt")
                for xb in range(XB_N):
                    # batched x stream: 4 contraction tiles per DMA
                    xt = xpool.tile([128, XB, 512], MM_DT, tag="xt", name="xt")
                    last_xt_dma = nc.sync.dma_start(xt[:], dram["xn"][sc, xb])
                    if xb == 1:
                        # rope table load sits behind the first x batch so it
                        # doesn't delay the very first matmuls
                        nc.sync.dma_start(rp[:], dram["rope"][sc])
                    if xb == 2 and sc > 0:
                        # previous chunk's v transposes, off the critical path
                        v_transposes(sc - 1)
                    for ki in range(XB):
                        kt = xb * XB + ki
                        first, last = kt == 0, kt == KT_N - 1
                        for h in range(NQH):
                            nc.tensor.matmul(
                                ps_q[h][:],
                                wq_sb[:, kt, h * 128:(h + 1) * 128],
                                xt[:, ki, :],
                                start=first,
                                stop=last,
                            )
                        nc.tensor.matmul(
                            ps_k[:], wk_sb[:, kt, :], xt[:, ki, :],
                            start=first, stop=last,
                        )
                        nc.tensor.matmul(
                            ps_vt[:], wv_sb[:, kt, :], xt[:, ki, :],
                            start=first, stop=last,
                        )

                cq, sq, ck, sk = rp[:, 0, :], rp[:, 1, :], rp[:, 2, :], rp[:, 3, :]
                last = sc == SC_N - 1
                if last:
                    # vtf copy + final v transposes first, so the PE stream
                    # runs straight into stage B; the DMA-swap rope keeps the
                    # rope entirely off the PE stream (nothing in early B
                    # depends on chunk 3's rope)
                    nc.vector.tensor_copy(vtf_all[:, sc, :], ps_vt[:])
                    v_transposes(sc)
                # q ropes first: they release the PSUM banks the next chunk's
                # first matmuls write
                for h in range(NQH):
                    rope_apply(qT_c[h][sc][:], ps_q[h][:], cq, sq, dma_swap=last)
                rope_apply(kT_c[sc][:], ps_k[:], ck, sk, dma_swap=last)
                if not last:
                    nc.vector.tensor_copy(vtf_all[:, sc, :], ps_vt[:])

        if "B" not in STAGES:
            # A-only debug: flush qT so the program has a live output
            with tc.tile_pool(name="dbgp", bufs=1) as dbgp:
                dbg = dbgp.tile([128, 2, 512], F32, tag="ob", name="dbg")
                nc.vector.tensor_copy(dbg[:, 0, :], qT_c[0][0][:])
                nc.sync.dma_start(out[0, 0], dbg[:])
            return
        # ---- stage B: attention (transposed scores), per (i-chunk, head) ----
        # wo prefetch into the SBUF freed by the stage-A weight pool (dep
        # edges keep the transfers behind the x stream); stage C then runs
        # with zero DMA waits.
        with tc.tile_pool(name="wo", bufs=1) as wopool, \
             tc.tile_pool(name="pexp", bufs=6) as pexpool, \
             tc.tile_pool(name="recip", bufs=2) as rcpool, \
             tc.tile_pool(name="outsb", bufs=4) as obpool, \
             tc.tile_pool(name="psB", bufs=1, space="PSUM") as psB:
            wo_sb = wopool.tile(
                [128, ECH_N, NQH, 512], MM_DT, tag="wo", name="wo_sb"
            )
            for ech in range(ECH_N):
                eng = nc.scalar if ech % 2 else nc.sync
                wdma = eng.dma_start(wo_sb[:, ech, :, :], dram["wo"][ech])
                if last_xt_dma is not None:
                    add_dep_helper(
                        wdma.ins, last_xt_dma.ins, sync=True,
                        reason="wo prefetch waits for x stream",
                    )

            for ic in range(SC_N):
                njt = 4 * (ic + 1) if causal else JT_N
                for h in range(NQH):
                    pv = psB.tile([128, 512], F32, tag="pspv", name="pspv", bufs=2)
                    rs = psB.tile([128, 512], F32, tag="psrs", name="psrs", bufs=2)
                    for jt in range(njt):
                        # diagonal tiles are mostly masked: only queries
                        # n >= 128*d attend to key tile 4*ic+d, so compute
                        # just that column range (identical math — the rest
                        # was zeroed after exp anyway)
                        d = jt - 4 * ic
                        c0 = 128 * d if (causal and d > 0) else 0
                        w = 512 - c0
                        st = psB.tile(
                            [128, 512], F32, tag="psst", name="psst", bufs=4
                        )
                        nc.tensor.matmul(
                            st[:, 0:w],
                            kT_c[jt // 4][:, (jt % 4) * 128:(jt % 4 + 1) * 128],
                            qT_c[h][ic][:, c0:512],
                            start=True,
                            stop=True,
                        )
                        pe = pexpool.tile([128, 512], MM_DT, tag="pe", name="pe")
                        nc.scalar.activation(
                            pe[:, 0:w], st[:, 0:w],
                            mybir.ActivationFunctionType.Exp,
                        )
                        if causal and d >= 0:
                            # keep where query (c0 + n') >= key (128*jt + p)
                            # relative to chunk: n' - p >= 0 in slice coords
                            nc.gpsimd.affine_select(
                                out=pe[:, 0:w],
                                in_=pe[:, 0:w],
                                pattern=[[1, w]],
                                compare_op=mybir.AluOpType.is_ge,
                                fill=0.0,
                                base=0,
                                channel_multiplier=-1,
                            )
                        first, last = jt == 0, jt == njt - 1
                        nc.tensor.matmul(
                            pv[:, c0:512], v_sb[:, jt, :], pe[:, 0:w],
                            start=first, stop=last,
                        )
                        nc.tensor.matmul(
                            rs[:, c0:512], ones_sb[:], pe[:, 0:w],
                            start=first, stop=last,
                        )
                    rc = rcpool.tile([128, 512], F32, tag="rc", name="rc")
                    nc.vector.reciprocal_approx_fast(rc[:], rs[:])
                    nc.vector.tensor_mul(qT_c[h][ic][:], pv[:], rc[:])

            if "C" not in STAGES:
                dbg = obpool.tile([128, 2, 512], F32, tag="ob", name="dbg")
                nc.vector.tensor_copy(dbg[:, 0, :], qT_c[0][0][:])
                nc.sync.dma_start(out[0, 0], dbg[:])
                return
            # ---- stage C: wo matmul (outT lives in qT_c) ----
            for ech in range(ECH_N):
                for itp in range(JT_N // 2):
                    # pack 2 row-tiles per output DMA (halves DMA issue count)
                    ob = obpool.tile([128, 2, 512], F32, tag="ob", name="ob")
                    for ii in range(2):
                        it = itp * 2 + ii
                        # share the st tag: 3 PSUM bufs decouple eviction
                        # from the next accumulation chain
                        pc = psB.tile(
                            [128, 512], F32, tag="psst", name="psc", bufs=4
                        )
                        for h in range(NQH):
                            nc.tensor.matmul(
                                pc[:],
                                qT_c[h][it // 4][
                                    :, (it % 4) * 128:(it % 4 + 1) * 128
                                ],
                                wo_sb[:, ech, h, :],
                                start=h == 0,
                                stop=h == NQH - 1,
                            )
                        nc.vector.tensor_copy(ob[:, ii, :], pc[:])
                    # alternate HWDGE rings so completion latency overlaps
                    eng = nc.scalar if itp % 2 else nc.sync
                    eng.dma_start(out[ech, itp], ob[:])


def build_nc(causal: bool = True, reps: int = 1):
    nc = bacc.Bacc(
        "TRN2", target_bir_lowering=False, debug=False, num_devices=N_CORES
    )
    dram = {}
    for name, shape, dt in [
        # host-prepermuted layouts: every DMA reads/writes contiguous
        # per-partition runs
        ("xn", [SC_N, XB_N, 128, XB, 512], MM_DT),
        ("wq", [128, KT_N, QCOLS], MM_DT),
        ("wk", [128, KT_N, HD], MM_DT),
        ("wv", [128, KT_N, HD], MM_DT),
        ("wo", [ECH_N, 128, NQH, 512], MM_DT),
        ("rope", [SC_N, HD, 4, 512], F32),
        ("ones", [128, 128], MM_DT),
        ("ident", [128, 128], MM_DT),
        ("rotm", [128, 128], MM_DT),
    ]:
        dram[name] = nc.dram_tensor(name, shape, dt, kind="ExternalInput").ap()
    out = nc.dram_tensor("out", [ECH_N, JT_N // 2, 128, 2, 512], F32,
                         kind="ExternalOutput").ap()

    with tile.TileContext(nc) as tc:
        for _ in range(reps):
            _emit_body(nc, tc, dram, out, causal)

    nc.compile()
    return nc


def get_nc(causal: bool = True):
    if causal not in _BUILD_CACHE:
        _BUILD_CACHE[causal] = build_nc(causal)
    return _BUILD_CACHE[causal]


def _mm_np(a):
    return np.ascontiguousarray(a).astype(mybir.dt.np(MM_DT))


def prep_in_maps(x, sincos, wq, wk, wv, wo):
    """Host-side shard + layout prep. Returns list of per-core input dicts.

    All tensors are pre-permuted so that every device DMA moves contiguous
    per-partition runs (device DMA engines are far more efficient that way).
    """
    x = np.asarray(x, np.float32)
    assert x.shape == (1, S, D)
    # xn[sc, xb, p, ki, n] = x[sc*512 + n, (xb*XB + ki)*128 + p]
    xn = _mm_np(
        x[0].reshape(SC_N, 512, XB_N, XB, 128).transpose(0, 2, 4, 3, 1)
    )

    sincos = np.asarray(sincos, np.float32)
    sinT = np.ascontiguousarray(sincos[:S, :HD].T)
    cosT = np.ascontiguousarray(sincos[:S, HD:].T)
    scale = np.float32(1.0 / math.sqrt(HD))
    # rope[sc, d, tbl, n], tbl order: cosq, sinq, cosk, sink; sin tables
    # carry the rotate-half sign (rows 0:64 negated), rotm is a plain perm
    sin_sgn = sinT.copy()
    sin_sgn[:64] = -sinT[:64]
    rope = np.stack(
        [cosT * scale, sin_sgn * scale, cosT, sin_sgn], axis=0
    ).reshape(4, HD, SC_N, 512).transpose(2, 1, 0, 3)
    rope = np.ascontiguousarray(rope)

    # rot(t)[m] = t[(m+64) % 128] as lhsT[k, m]; the sign lives in the
    # sin tables so the DMA-swap and PE-matmul rope variants match
    rotm = np.zeros((128, 128), np.float32)
    for m in range(64):
        rotm[m + 64, m] = 1.0
        rotm[m, m + 64] = 1.0

    wq = np.asarray(wq, np.float32)
    wk = np.asarray(wk, np.float32)
    wv = np.asarray(wv, np.float32)
    wo = np.asarray(wo, np.float32)

    in_maps = []
    for c in range(N_CORES):
        wq_c = wq[:, c * QCOLS:(c + 1) * QCOLS]          # [D, 512]
        wk_c = wk[:, c * HD:(c + 1) * HD]                # [D, 128]
        wv_c = wv[:, c * HD:(c + 1) * HD]
        wo_c = wo[c * QCOLS:(c + 1) * QCOLS, :]          # [512, D]
        in_maps.append(
            {
                "xn": xn,
                # wq[p, kt, m] = wq_c[kt*128 + p, m]
                "wq": _mm_np(
                    wq_c.reshape(KT_N, 128, QCOLS).transpose(1, 0, 2)
                ),
                "wk": _mm_np(
                    wk_c.reshape(KT_N, 128, HD).transpose(1, 0, 2)
                ),
                "wv": _mm_np(
                    wv_c.reshape(KT_N, 128, HD).transpose(1, 0, 2)
                ),
                # wo[ech, p, a, n] = wo_c[a*128 + p, ech*512 + n]
                "wo": _mm_np(
                    wo_c.reshape(NQH, 128, ECH_N, 512).transpose(2, 1, 0, 3)
                ),
                "rope": rope,
                "ones": _mm_np(np.ones((128, 128), np.float32)),
                "ident": _mm_np(np.eye(128, dtype=np.float32)),
                "rotm": _mm_np(rotm),
            }
        )
    return in_maps


def unpermute_out(out_n):
    """out_n [ech, itp, p, ii, n] -> out [S, D]."""
    return np.ascontiguousarray(
        out_n.transpose(1, 3, 2, 0, 4).reshape(S, D)
    )


def check_mask(full_causal_mask, start_pos) -> bool:
    """Returns True for causal (tril) mask, False for all-allowed."""
    sp = int(start_pos)
    assert sp == 0, f"kernel specialized for start_pos=0, got {sp}"
    m = np.asarray(full_causal_mask)
    assert m.shape == (1, 1, S, S)
    m = m[0, 0]
    tril = np.tril(np.ones((S, S), dtype=bool))
    if (m == tril).all():
        return True
    if m.all():
        return False
    raise AssertionError("unsupported mask pattern")


def kernel(
    x,
    start_pos,
    sincos,
    full_causal_mask,
    wq,
    wk,
    wv,
    wo,
    cache_k,
    cache_v,
):
    causal = check_mask(full_causal_mask, start_pos)
    # cache_k/cache_v are zero and fully overwritten in the attended region
    # (start_pos=0, seq_len == max_seq_len) — they do not affect the output.
    nc = get_nc(causal)
    in_maps = prep_in_maps(x, sincos, wq, wk, wv, wo)
    res = run_bass_kernel_spmd(nc, in_maps, list(range(N_CORES)))
    acc = res.results[0]["out"].astype(np.float32)
    for c in range(1, N_CORES):
        acc = acc + res.results[c]["out"]
    return unpermute_out(acc)[np.newaxis]


# revision 15
# speedup vs baseline: 1.0569x; 1.0155x over previous
"""Tensor-parallel GQA attention (Llama-3-8B shape, prefill, start_pos=0) on 8
Trainium2 NeuronCores.

Sharding: core i owns kv-head i and q-heads 4i..4i+3 — wq/wk/wv column-shards,
wo row-shard, x replicated.  Each core computes a partial [2048, 4096] output
(its heads pushed through its wo rows); the host sums the 8 partials
(all-reduce equivalent).

Per-core kernel layout (matmuls N=512, bf16 operands, fp32 PSUM accumulate):
  - xT [D, S] on device; projections computed with weights as the stationary
    operand, giving qT/kT/vT in [head_dim, seq] layout directly.
  - RoPE in [head_dim, seq] layout; rotate-half is a PE matmul against a
    signed permutation matrix (keeps the DVE FIFO free of DMA waits), the
    1/sqrt(hd) score scale is folded into q's sin/cos tables.
  - Scores computed transposed, ST[j, i] = kT.T @ qT, so exp needs no
    transposes and PV consumes Pexp^T directly (lhsT = v tile [j, d],
    rhs = Pexp^T [j, i] -> outT [d, i] accumulated over j-tiles in PSUM).
  - No max-subtraction in softmax (scores bounded); row sums come from an
    all-ones stationary matmul accumulated alongside PV, so normalization is
    one approx-reciprocal + one multiply.  outT overwrites qT storage.
  - Causal masking: only j-tiles with j <= chunk max are computed; the 4
    diagonal tiles per (head, i-chunk) get affine_select(fill=0) after exp.
  - wo matmul with lhsT = normalized outT slices, accumulated over the 4
    heads in PSUM; eviction is a plain copy, DMA'd to the partial output.

Scheduling notes (v5):
  - bf16 matmul operands: enables the fast-weight-load path (FWL is disabled
    for fp32), so LDWEIGHTS hides behind the previous matmul; halves DMA.
  - kT/qT live as per-chunk tiles so stage B's first scores depend only on
    chunk 0's rope, not the whole stage-A tail.
  - Two HWDGE issue queues: weights on the scalar-engine ring; x stream,
    rope tables and output stores on the sync-engine ring.  Weight loads are
    interleaved in PE-consumption order so the first matmul starts ~4 us in.
  - x streamed in 512 KB batches (4 k-tiles), quad-buffered.
  - RoPE eviction copy is the only read of the accumulation PSUM bank, and
    q-head ropes run before k, so the banks the next chunk's first matmuls
    need are released early.
  - v transposes for chunk sc are emitted mid-chunk sc+1, off the PE
    critical path.
  - Stage B pv/rs PSUM tiles double-buffered, st triple-buffered; softmax
    reciprocal is the fast approx DVE op.
  - All of wo is prefetched on both rings once the x stream has issued (dep
    edges keep it out of stage A's bandwidth); stage C has zero DMA waits,
    and its PSUM accumulators share the st tag (3 bufs).
"""

import math
from contextlib import ExitStack

import numpy as np

import concourse.bass as bass
import concourse.tile as tile
from concourse import bacc, mybir
from concourse.bass_utils import run_bass_kernel_spmd
from concourse.tile import add_dep_helper

# ---- problem shape (hardcoded per contract) ----
S = 2048           # seq len
D = 4096           # model dim
HD = 128           # head dim
N_CORES = 8
NQH = 4            # q heads per core
QCOLS = NQH * HD   # 512 wq columns per core
SC_N = 4           # seq chunks of 512
KT_N = D // 128    # 32 contraction tiles
JT_N = S // 128    # 16 key tiles
ECH_N = D // 512   # 8 output column chunks
XB = 4             # x k-tiles per DMA batch
XB_N = KT_N // XB  # 8 batches per seq chunk

F32 = mybir.dt.float32
F32R = mybir.dt.float32r
BF16 = mybir.dt.bfloat16

MM_DT = BF16          # matmul operand dtype: F32R (accurate) or BF16 (fast)

_BUILD_CACHE: dict = {}

STAGES = "ABC"  # debug knob: subset of stages to emit


def _emit_body(nc, tc, dram, out, causal: bool):
    with ExitStack() as ctx:
        def pool(name, bufs, space="SBUF"):
            return ctx.enter_context(tc.tile_pool(name=name, bufs=bufs, space=space))

        pools = {
            "persist": pool("persist", 1),
            "rope_tmp": pool("rope_tmp", 3),
        }
        persist = pools["persist"]

        # persistent activations, split per chunk so stage B's dependencies
        # are per-chunk, not whole-tensor
        kT_c = [
            persist.tile([128, 512], MM_DT, tag=f"kT{sc}", name=f"kT{sc}")
            for sc in range(SC_N)
        ]
        # qT doubles as outT: B(h, ic) consumes qT[h][ic] then writes the
        # normalized attention output into the same tile.
        qT_c = [
            [
                persist.tile([128, 512], MM_DT, tag=f"qT{h}_{sc}", name=f"qT{h}_{sc}")
                for sc in range(SC_N)
            ]
            for h in range(NQH)
        ]
        v_sb = persist.tile([128, JT_N, HD], MM_DT, tag="v", name="v_sb")
        ones_sb = persist.tile([128, 128], MM_DT, tag="ones", name="ones_sb")
        ident_sb = persist.tile([128, 128], MM_DT, tag="ident", name="ident_sb")
        rotm_sb = persist.tile([128, 128], MM_DT, tag="rotm", name="rotm_sb")

        last_xt_dma = None

        # ---- stage A: projections + RoPE, per 512-wide seq chunk ----
        with tc.tile_pool(name="w", bufs=1) as wpool, \
             tc.tile_pool(name="x", bufs=6) as xpool, \
             tc.tile_pool(name="rope_in", bufs=2) as rpool, \
             tc.tile_pool(name="vtf", bufs=1) as vtfpool, \
             tc.tile_pool(name="psA", bufs=1, space="PSUM") as psA:
            # resident weights, loaded in PE-consumption order: slices of 4
            # k-tiles for each of wq/wk/wv so the first matmuls start early
            wq_sb = wpool.tile([128, KT_N, QCOLS], MM_DT, tag="wq", name="wq_sb")
            wk_sb = wpool.tile([128, KT_N, HD], MM_DT, tag="wk", name="wk_sb")
            wv_sb = wpool.tile([128, KT_N, HD], MM_DT, tag="wv", name="wv_sb")
            wslices = [slice(0, 1), slice(1, XB)] + [
                slice(g * XB, (g + 1) * XB) for g in range(1, XB_N)
            ]
            for g, gs in enumerate(wslices):
                nc.scalar.dma_start(wq_sb[:, gs, :], dram["wq"][:, gs, :])
                nc.scalar.dma_start(wk_sb[:, gs, :], dram["wk"][:, gs, :])
                nc.scalar.dma_start(wv_sb[:, gs, :], dram["wv"][:, gs, :])
                if g == 0:
                    nc.scalar.dma_start(rotm_sb[:], dram["rotm"][:])
                if g == 2:
                    nc.scalar.dma_start(ident_sb[:], dram["ident"][:])
                    nc.scalar.dma_start(ones_sb[:], dram["ones"][:])

            vtf_all = vtfpool.tile(
                [128, SC_N, 512], MM_DT, tag="vtf", name="vtf_all"
            )

            def v_transposes(sc):
                for vi in range(4):
                    ptr = psA.tile(
                        [128, 128], MM_DT, tag="pstr", name="pstr", bufs=2
                    )
                    nc.tensor.transpose(
                        ptr[:],
                        vtf_all[:, sc, vi * 128:(vi + 1) * 128],
                        ident_sb[:],
                    )
                    nc.vector.tensor_copy(v_sb[:, sc * 4 + vi, :], ptr[:])

            def rope_apply(dst, ps, cos_t, sin_t, dma_swap=False):
                """dst = ps * cos + rot(ps) * sin_signed.

                rot is a PE matmul against a plain permutation (keeps the DVE
                FIFO free of DMA waits); with dma_swap=True it is two
                SBUF->SBUF DMAs instead, keeping the PE stream free — used
                for the last chunk so stage B's scores aren't stuck behind
                rope matmuls nothing depends on yet.  The eviction copy is
                the only read of the accumulation bank, so it is released as
                soon as the copy completes."""
                qc = pools["rope_tmp"].tile(
                    [128, 512], MM_DT, tag="rope_qc", name="rqc"
                )
                nc.vector.tensor_copy(qc[:], ps)
                tc_ = pools["rope_tmp"].tile(
                    [128, 512], F32, tag="rope_tc", name="rtc"
                )
                qs = pools["rope_tmp"].tile(
                    [128, 512], F32, tag="rope_qs", name="rqs"
                )
                if dma_swap:
                    qsw = pools["rope_tmp"].tile(
                        [128, 512], MM_DT, tag="rope_qsw", name="rqsw"
                    )
                    nc.scalar.dma_start(qsw[0:64, :], qc[64:128, :])
                    nc.scalar.dma_start(qsw[64:128, :], qc[0:64, :])
                    nc.vector.tensor_mul(tc_[:], qc[:], cos_t)
                    nc.vector.tensor_mul(qs[:], qsw[:], sin_t)
                else:
                    sw = psA.tile(
                        [128, 512], F32, tag="pstr", name="pssw", bufs=2
                    )
                    nc.tensor.matmul(
                        sw[:], rotm_sb[:], qc[:], start=True, stop=True
                    )
                    nc.vector.tensor_mul(tc_[:], qc[:], cos_t)
                    nc.vector.tensor_mul(qs[:], sw[:], sin_t)
                nc.vector.tensor_add(dst, tc_[:], qs[:])

            for sc in range(SC_N):
                rp = rpool.tile([HD, 4, 512], F32, tag="rp", name="rp")
                ps_q = [
                    psA.tile([128, 512], F32, tag=f"psq{h}", name=f"psq{h}")
                    for h in range(NQH)
                ]
                ps_k = psA.tile([128, 512], F32, tag="psk", name="psk")
                ps_vt = psA.tile([128, 512], F32, tag="psvt", name="psvt")
                for xb in range(XB_N):
                    # batched x stream: 4 contraction tiles per DMA
                    xt = xpool.tile([128, XB, 512], MM_DT, tag="xt", name="xt")
                    last_xt_dma = nc.sync.dma_start(xt[:], dram["xn"][sc, xb])
                    if xb == 1:
                        # rope table load sits behind the first x batch so it
                        # doesn't delay the very first matmuls
                        nc.sync.dma_start(rp[:], dram["rope"][sc])
                    if xb == 2 and sc > 0:
                        # previous chunk's v transposes, off the critical path
                        v_transposes(sc - 1)
                    for ki in range(XB):
                        kt = xb * XB + ki
                        first, last = kt == 0, kt == KT_N - 1
                        for h in range(NQH):
                            nc.tensor.matmul(
                                ps_q[h][:],
                                wq_sb[:, kt, h * 128:(h + 1) * 128],
                                xt[:, ki, :],
                                start=first,
                                stop=last,
                            )
                        nc.tensor.matmul(
                            ps_k[:], wk_sb[:, kt, :], xt[:, ki, :],
                            start=first, stop=last,
                        )
                        nc.tensor.matmul(
                            ps_vt[:], wv_sb[:, kt, :], xt[:, ki, :],
                            start=first, stop=last,
                        )

                cq, sq, ck, sk = rp[:, 0, :], rp[:, 1, :], rp[:, 2, :], rp[:, 3, :]
                last = sc == SC_N - 1
                if last:
                    # vtf copy + final v transposes first, so the PE stream
                    # runs straight into stage B (nothing in early B depends
                    # on chunk 3's rope)
                    nc.vector.tensor_copy(vtf_all[:, sc, :], ps_vt[:])
                    v_transposes(sc)
                # q ropes first: they release the PSUM banks the next chunk's
                # first matmuls write
                for h in range(NQH):
                    rope_apply(qT_c[h][sc][:], ps_q[h][:], cq, sq)
                rope_apply(kT_c[sc][:], ps_k[:], ck, sk)
                if not last:
                    nc.vector.tensor_copy(vtf_all[:, sc, :], ps_vt[:])

        if "B" not in STAGES:
            # A-only debug: flush qT so the program has a live output
            with tc.tile_pool(name="dbgp", bufs=1) as dbgp:
                dbg = dbgp.tile([128, 2, 512], F32, tag="ob", name="dbg")
                nc.vector.tensor_copy(dbg[:, 0, :], qT_c[0][0][:])
                nc.sync.dma_start(out[0, 0], dbg[:])
            return
        # ---- stage B: attention (transposed scores), per (i-chunk, head) ----
        # wo prefetch into the SBUF freed by the stage-A weight pool (dep
        # edges keep the transfers behind the x stream); stage C then runs
        # with zero DMA waits.
        with tc.tile_pool(name="wo", bufs=1) as wopool, \
             tc.tile_pool(name="pexp", bufs=6) as pexpool, \
             tc.tile_pool(name="recip", bufs=2) as rcpool, \
             tc.tile_pool(name="outsb", bufs=4) as obpool, \
             tc.tile_pool(name="psB", bufs=1, space="PSUM") as psB:
            wo_sb = wopool.tile(
                [128, ECH_N, NQH, 512], MM_DT, tag="wo", name="wo_sb"
            )
            for ech in range(ECH_N):
                eng = nc.scalar if ech % 2 else nc.sync
                wdma = eng.dma_start(wo_sb[:, ech, :, :], dram["wo"][ech])
                if last_xt_dma is not None:
                    add_dep_helper(
                        wdma.ins, last_xt_dma.ins, sync=True,
                        reason="wo prefetch waits for x stream",
                    )

            for ic in range(SC_N):
                njt = 4 * (ic + 1) if causal else JT_N
                for h in range(NQH):
                    pv = psB.tile([128, 512], F32, tag="pspv", name="pspv", bufs=2)
                    rs = psB.tile([128, 512], F32, tag="psrs", name="psrs", bufs=2)
                    for jt in range(njt):
                        # diagonal tiles are mostly masked: only queries
                        # n >= 128*d attend to key tile 4*ic+d, so compute
                        # just that column range (identical math — the rest
                        # was zeroed after exp anyway)
                        d = jt - 4 * ic
                        c0 = 128 * d if (causal and d > 0) else 0
                        w = 512 - c0
                        st = psB.tile(
                            [128, 512], F32, tag="psst", name="psst", bufs=4
                        )
                        nc.tensor.matmul(
                            st[:, 0:w],
                            kT_c[jt // 4][:, (jt % 4) * 128:(jt % 4 + 1) * 128],
                            qT_c[h][ic][:, c0:512],
                            start=True,
                            stop=True,
                        )
                        pe = pexpool.tile([128, 512], MM_DT, tag="pe", name="pe")
                        nc.scalar.activation(
                            pe[:, 0:w], st[:, 0:w],
                            mybir.ActivationFunctionType.Exp,
                        )
                        if causal and d >= 0:
                            # keep where query (c0 + n') >= key (128*jt + p)
                            # relative to chunk: n' - p >= 0 in slice coords
                            nc.gpsimd.affine_select(
                                out=pe[:, 0:w],
                                in_=pe[:, 0:w],
                                pattern=[[1, w]],
                                compare_op=mybir.AluOpType.is_ge,
                                fill=0.0,
                                base=0,
                                channel_multiplier=-1,
                            )
                        first, last = jt == 0, jt == njt - 1
                        nc.tensor.matmul(
                            pv[:, c0:512], v_sb[:, jt, :], pe[:, 0:w],
                            start=first, stop=last,
                        )
                        nc.tensor.matmul(
                            rs[:, c0:512], ones_sb[:], pe[:, 0:w],
                            start=first, stop=last,
                        )
                    rc = rcpool.tile([128, 512], F32, tag="rc", name="rc")
                    nc.vector.reciprocal_approx_fast(rc[:], rs[:])
                    nc.vector.tensor_mul(qT_c[h][ic][:], pv[:], rc[:])

            if "C" not in STAGES:
                dbg = obpool.tile([128, 2, 512], F32, tag="ob", name="dbg")
                nc.vector.tensor_copy(dbg[:, 0, :], qT_c[0][0][:])
                nc.sync.dma_start(out[0, 0], dbg[:])
                return
            # ---- stage C: wo matmul (outT lives in qT_c) ----
            for ech in range(ECH_N):
                for itp in range(JT_N // 2):
                    # pack 2 row-tiles per output DMA (halves DMA issue count)
                    ob = obpool.tile([128, 2, 512], F32, tag="ob", name="ob")
                    for ii in range(2):
                        it = itp * 2 + ii
                        # share the st tag: 3 PSUM bufs decouple eviction
                        # from the next accumulation chain
                        pc = psB.tile(
                            [128, 512], F32, tag="psst", name="psc", bufs=4
                        )
                        for h in range(NQH):
                            nc.tensor.matmul(
                                pc[:],
                                qT_c[h][it // 4][
                                    :, (it % 4) * 128:(it % 4 + 1) * 128
                                ],
                                wo_sb[:, ech, h, :],
                                start=h == 0,
                                stop=h == NQH - 1,
                            )
                        nc.vector.tensor_copy(ob[:, ii, :], pc[:])
                    # alternate HWDGE rings so completion latency overlaps
                    eng = nc.scalar if itp % 2 else nc.sync
                    eng.dma_start(out[ech, itp], ob[:])


def build_nc(causal: bool = True, reps: int = 1):
    nc = bacc.Bacc(
        "TRN2", target_bir_lowering=False, debug=False, num_devices=N_CORES
    )
    dram = {}
    for name, shape, dt in [
        # host-prepermuted layouts: every DMA reads/writes contiguous
        # per-partition runs
        ("xn", [SC_N, XB_N, 128, XB, 512], MM_DT),
        ("wq", [128, KT_N, QCOLS], MM_DT),
        ("wk", [128, KT_N, HD], MM_DT),
        ("wv", [128, KT_N, HD], MM_DT),
        ("wo", [ECH_N, 128, NQH, 512], MM_DT),
        ("rope", [SC_N, HD, 4, 512], F32),
        ("ones", [128, 128], MM_DT),
        ("ident", [128, 128], MM_DT),
        ("rotm", [128, 128], MM_DT),
    ]:
        dram[name] = nc.dram_tensor(name, shape, dt, kind="ExternalInput").ap()
    out = nc.dram_tensor("out", [ECH_N, JT_N // 2, 128, 2, 512], F32,
                         kind="ExternalOutput").ap()

    with tile.TileContext(nc) as tc:
        for _ in range(reps):
            _emit_body(nc, tc, dram, out, causal)

    nc.compile()
    return nc


def get_nc(causal: bool = True):
    if causal not in _BUILD_CACHE:
        _BUILD_CACHE[causal] = build_nc(causal)
    return _BUILD_CACHE[causal]


def _mm_np(a):
    return np.ascontiguousarray(a).astype(mybir.dt.np(MM_DT))


def prep_in_maps(x, sincos, wq, wk, wv, wo):
    """Host-side shard + layout prep. Returns list of per-core input dicts.

    All tensors are pre-permuted so that every device DMA moves contiguous
    per-partition runs (device DMA engines are far more efficient that way).
    """
    x = np.asarray(x, np.float32)
    assert x.shape == (1, S, D)
    # xn[sc, xb, p, ki, n] = x[sc*512 + n, (xb*XB + ki)*128 + p]
    xn = _mm_np(
        x[0].reshape(SC_N, 512, XB_N, XB, 128).transpose(0, 2, 4, 3, 1)
    )

    sincos = np.asarray(sincos, np.float32)
    sinT = np.ascontiguousarray(sincos[:S, :HD].T)
    cosT = np.ascontiguousarray(sincos[:S, HD:].T)
    scale = np.float32(1.0 / math.sqrt(HD))
    # rope[sc, d, tbl, n], tbl order: cosq, sinq, cosk, sink; sin tables
    # carry the rotate-half sign (rows 0:64 negated), rotm is a plain perm
    sin_sgn = sinT.copy()
    sin_sgn[:64] = -sinT[:64]
    rope = np.stack(
        [cosT * scale, sin_sgn * scale, cosT, sin_sgn], axis=0
    ).reshape(4, HD, SC_N, 512).transpose(2, 1, 0, 3)
    rope = np.ascontiguousarray(rope)

    # rot(t)[m] = t[(m+64) % 128] as lhsT[k, m]; the sign lives in the
    # sin tables so the DMA-swap and PE-matmul rope variants match
    rotm = np.zeros((128, 128), np.float32)
    for m in range(64):
        rotm[m + 64, m] = 1.0
        rotm[m, m + 64] = 1.0

    wq = np.asarray(wq, np.float32)
    wk = np.asarray(wk, np.float32)
    wv = np.asarray(wv, np.float32)
    wo = np.asarray(wo, np.float32)

    in_maps = []
    for c in range(N_CORES):
        wq_c = wq[:, c * QCOLS:(c + 1) * QCOLS]          # [D, 512]
        wk_c = wk[:, c * HD:(c + 1) * HD]                # [D, 128]
        wv_c = wv[:, c * HD:(c + 1) * HD]
        wo_c = wo[c * QCOLS:(c + 1) * QCOLS, :]          # [512, D]
        in_maps.append(
            {
                "xn": xn,
                # wq[p, kt, m] = wq_c[kt*128 + p, m]
                "wq": _mm_np(
                    wq_c.reshape(KT_N, 128, QCOLS).transpose(1, 0, 2)
                ),
                "wk": _mm_np(
                    wk_c.reshape(KT_N, 128, HD).transpose(1, 0, 2)
                ),
                "wv": _mm_np(
                    wv_c.reshape(KT_N, 128, HD).transpose(1, 0, 2)
                ),
                # wo[ech, p, a, n] = wo_c[a*128 + p, ech*512 + n]
                "wo": _mm_np(
                    wo_c.reshape(NQH, 128, ECH_N, 512).transpose(2, 1, 0, 3)
                ),
                "rope": rope,
                "ones": _mm_np(np.ones((128, 128), np.float32)),
                "ident": _mm_np(np.eye(128, dtype=np.float32)),
                "rotm": _mm_np(rotm),
            }
        )
    return in_maps


def unpermute_out(out_n):
    """out_n [ech, itp, p, ii, n] -> out [S, D]."""
    return np.ascontiguousarray(
        out_n.transpose(1, 3, 2, 0, 4).reshape(S, D)
    )


def check_mask(full_causal_mask, start_pos) -> bool:
    """Returns True for causal (tril) mask, False for all-allowed."""
    sp = int(start_pos)
    assert sp == 0, f"kernel specialized for start_pos=0, got {sp}"
    m = np.asarray(full_causal_mask)
    assert m.shape == (1, 1, S, S)
    m = m[0, 0]
    tril = np.tril(np.ones((S, S), dtype=bool))
    if (m == tril).all():
        return True
    if m.all():
        return False
    raise AssertionError("unsupported mask pattern")


def kernel(
    x,
    start_pos,
    sincos,
    full_causal_mask,
    wq,
    wk,
    wv,
    wo,
    cache_k,
    cache_v,
):
    causal = check_mask(full_causal_mask, start_pos)
    # cache_k/cache_v are zero and fully overwritten in the attended region
    # (start_pos=0, seq_len == max_seq_len) — they do not affect the output.
    nc = get_nc(causal)
    in_maps = prep_in_maps(x, sincos, wq, wk, wv, wo)
    res = run_bass_kernel_spmd(nc, in_maps, list(range(N_CORES)))
    acc = res.results[0]["out"].astype(np.float32)
    for c in range(1, N_CORES):
        acc = acc + res.results[c]["out"]
    return unpermute_out(acc)[np.newaxis]
